# revision 17
# baseline (speedup 1.0000x reference)
"""Trainium2 Bass kernel for nn_CategoryAdder (embedding lookup + masked add).

Computation: out[b,s,:] = inputs[b,s,:] + emb where
  emb = table[categories[b,s]] masked to zero when categories[b,s]==0 or
  s == mask_positions[b].

Host-side preprocessing folds both masks into the data:
  - categories[b, mask_positions[b]] = 0
  - table row 0 zeroed (on a copy)
so the device computes exactly: out = inputs + table0[categories].

Sharding: data-parallel over batch across 8 NeuronCores (8 batches per core,
16384 tokens/core). Table replicated.

The kernel is HBM-bandwidth bound (~358 GB/s/core), so the embedding lookup
is reorganized to avoid per-token DMA-gather traffic (32 MB/core of random
2 KB reads + ~165 us of serial Q7 descriptor generation):

  - The host sorts each core's tokens by category and bins them into 40
    value-buckets (cat//128). The first 384 tokens of bucket k fill three
    128-token "main" groups whose table rows all come from the static
    128-row table chunk k. TensorE computes each group's embeddings as
    onehotT[128 lo x 128 tok] @ chunk[128 lo x 512] into PSUM — a gather
    with zero DMA bytes (table sits in SBUF, 5 MB loaded once).
  - Bucket overflow tokens (~1390 of 16384, capacity 2048 = +5 sigma) go
    to a stray region serviced by one SWDGE dma_gather (trailing -1 indices
    are skipped, so only the real strays are fetched).
  - DVE adds x (bf16, from HBM in host-permuted slot-major order so every
    DMA is a fat contiguous per-partition read) to PSUM and writes bf16.

x, table rows and the output travel as bf16 and the one-hot as fp8 (exact
0/1); the harness gate is 2e-2 relative and the bf16 rounding of x, emb and
out contributes 2.45e-3. Per-core HBM traffic: 16.8 MB x + 16.8 MB out +
5.2 MB table + 1.9 MB one-hot + ~1.4 MB stray gather ~= 42 MB vs 96 MB for
the naive all-gather pipeline. Loads and stores are split across the two
HWDGE rings (nc.sync / nc.scalar); x/out tile pools are 10 deep so DMA
dispatch runs well ahead of the DVE adds that pace the steady state.

The program is identical across cores and inputs (static schedule); only the
input data varies. Host pre/post-processing is pure index bookkeeping plus
permutation of the I/O arrays.
"""

import ml_dtypes
import numpy as np

import concourse.mybir as mybir
from concourse import bacc, tile
from concourse.bass_utils import run_bass_kernel_spmd

BF16 = ml_dtypes.bfloat16


def _ensure_axon_ntff_hook_module():
    """run_bass_kernel_spmd(trace=True) under axon imports antenv.axon_hooks,
    which this image lacks — install a fallback shim (backed by the boot
    module's ctypes hook when available) so a BASS_TRACE=1 environment does
    not crash the kernel. No-op when the real module exists."""
    try:
        import antenv.axon_hooks  # noqa: F401
        return
    except ImportError:
        pass
    import sys
    import types

    hook = None
    try:
        import trn_agent_boot.trn_boot as _tb

        hook = _tb._ntff_profile_via_ctypes("/opt/axon/libaxon_pjrt.so")
    except Exception:
        hook = None  # get_..._hook() -> None makes bass_utils skip tracing
    mod = types.ModuleType("antenv.axon_hooks")
    mod.get_axon_ntff_profile_hook = lambda: hook
    mod.set_axon_ntff_profile_hook = lambda h: None
    sys.modules["antenv.axon_hooks"] = mod


_ensure_axon_ntff_hook_module()

B, S, D = 64, 2048, 512
N_CAT = 5000
N_CORES = 8
B_PER = B // N_CORES          # 8 batches per core
NTOK = B_PER * S              # 16384 tokens per core

N_CHUNK = 40                  # 128-row table chunks (5120 padded rows)
N_CATP = N_CHUNK * 128        # 5120
GP_BUCKET = 3                 # main groups per bucket (bucket 39: one group)
BUCKET_CAP = GP_BUCKET * 128  # 384 main tokens per bucket 0..38
MAIN_GROUPS = 39 * GP_BUCKET + 1    # 118 (bucket 39 holds cats 4992..4999)
MAIN_SLOTS = MAIN_GROUPS * 128      # 15104
STRAY_GROUPS = 16
STRAY_CAP = STRAY_GROUPS * 128      # 2048
G_TOT = MAIN_GROUPS + STRAY_GROUPS  # 136 column-chunks of x/out
SLOTS = G_TOT * 128                 # 17408
ST = 4                        # groups per supertile (4 PSUM banks)
N_SST = (MAIN_GROUPS + ST - 1) // ST   # 30 (29 full + one of 2)
N_SST_S = STRAY_GROUPS // ST  # 4 stray supertiles
IDX_COLS = STRAY_CAP // 16    # 128


def _build_nc():
    nc = bacc.Bacc("TRN2", target_bir_lowering=False, debug=False)
    x = nc.dram_tensor("x", [128, G_TOT, D], mybir.dt.bfloat16, kind="ExternalInput")
    tblc = nc.dram_tensor("tblc", [128, N_CHUNK * D], mybir.dt.bfloat16,
                          kind="ExternalInput")
    tblr = nc.dram_tensor("tblr", [N_CATP, D], mybir.dt.bfloat16,
                          kind="ExternalInput")
    oh = nc.dram_tensor("oh", [128, MAIN_SLOTS], mybir.dt.float8e4,
                        kind="ExternalInput")
    idx = nc.dram_tensor("idx", [128, IDX_COLS], mybir.dt.int16,
                         kind="ExternalInput")
    out = nc.dram_tensor("out", [128, G_TOT, D], mybir.dt.bfloat16,
                         kind="ExternalOutput")

    with tile.TileContext(nc) as tc:
        with (
            tc.tile_pool(name="idxp", bufs=1) as idxp,
            tc.tile_pool(name="tblp", bufs=1) as tblp,
            tc.tile_pool(name="ohp", bufs=1) as ohp,
            tc.tile_pool(name="strayp", bufs=1) as strayp,
            tc.tile_pool(name="xp", bufs=10) as xp,
            tc.tile_pool(name="outp", bufs=10) as outp,
            tc.tile_pool(name="psp", bufs=2, space=bacc.bass.MemorySpace.PSUM) as psp,
        ):
            # Table chunks and one-hot matrices, resident in SBUF, loaded
            # in slices interleaved with the first x loads so the pipeline
            # fills fast (Tile tracks slice-granular dependencies). The
            # stray gather's idx load is deferred behind x1 so its ~18 us
            # of Q7 descriptor generation and its random-read drain stay
            # out of the fill window (done long before the stray adds).
            idx_sb = idxp.tile([128, IDX_COLS], mybir.dt.int16)
            semb = strayp.tile([128, STRAY_GROUPS * D], mybir.dt.bfloat16)
            tbl_sb = tblp.tile([128, N_CHUNK * D], mybir.dt.bfloat16)
            for i in range(5):
                sl = slice(i * 8 * D, (i + 1) * 8 * D)
                nc.scalar.dma_start(out=tbl_sb[:, sl], in_=tblc[:, sl])
            oh_sb = ohp.tile([128, MAIN_SLOTS], mybir.dt.float8e4)
            OH_SLICES = [(0, 30 * 128), (30 * 128, 60 * 128),
                         (60 * 128, 90 * 128), (90 * 128, MAIN_SLOTS)]
            sl = slice(*OH_SLICES[0])
            nc.sync.dma_start(out=oh_sb[:, sl], in_=oh[:, sl])

            for st in range(N_SST):
                if st == 1:
                    nc.sync.dma_start(out=idx_sb[:], in_=idx[:, :])
                    nc.gpsimd.dma_gather(
                        semb[:].rearrange("p (c e) -> p c e", e=D),
                        tblr[:, :],
                        idx_sb[:, :],
                        STRAY_CAP,
                        STRAY_CAP,
                        D,
                        single_packet=False,
                    )
                if st in (2, 4, 6):
                    sl = slice(*OH_SLICES[st // 2])
                    nc.sync.dma_start(out=oh_sb[:, sl], in_=oh[:, sl])
                g0 = st * ST
                n_g = min(ST, MAIN_GROUPS - g0)
                x_t = xp.tile([128, n_g * D], mybir.dt.bfloat16, tag="x")
                nc.sync.dma_start(
                    out=x_t[:],
                    in_=x[:, g0 : g0 + n_g, :].rearrange("p g e -> p (g e)"),
                )
                ps = psp.tile([128, n_g, D], mybir.dt.float32, tag="ps")
                for j in range(n_g):
                    g = g0 + j
                    k = g // GP_BUCKET if g < 117 else 39
                    nc.tensor.matmul(
                        ps[:, j, :],
                        oh_sb[:, g * 128 : (g + 1) * 128],
                        tbl_sb[:, k * D : (k + 1) * D],
                        start=True,
                        stop=True,
                    )
                o_t = outp.tile([128, n_g * D], mybir.dt.bfloat16, tag="o")
                nc.vector.tensor_add(
                    out=o_t[:],
                    in0=x_t[:],
                    in1=ps[:].rearrange("p g e -> p (g e)"),
                )
                nc.scalar.dma_start(
                    out=out[:, g0 : g0 + n_g, :].rearrange("p g e -> p (g e)"),
                    in_=o_t[:],
                )

            # Stray region: x + gathered emb, through the same supertile
            # pipeline (reuses the x/out pools and tile shapes).
            for st in range(N_SST_S):
                g0 = MAIN_GROUPS + st * ST
                sx = xp.tile([128, ST * D], mybir.dt.bfloat16, tag="x")
                nc.sync.dma_start(
                    out=sx[:],
                    in_=x[:, g0 : g0 + ST, :].rearrange("p g e -> p (g e)"),
                )
                so = outp.tile([128, ST * D], mybir.dt.bfloat16, tag="o")
                nc.vector.tensor_add(
                    out=so[:],
                    in0=sx[:],
                    in1=semb[:, st * ST * D : (st + 1) * ST * D],
                )
                nc.scalar.dma_start(
                    out=out[:, g0 : g0 + ST, :].rearrange("p g e -> p (g e)"),
                    in_=so[:],
                )
    nc.compile()
    return nc


def _prep_core(x_shard: np.ndarray, cat_shard: np.ndarray):
    """Sort one core's tokens by category into the static slot layout.

    Returns (x_perm [128,130,512] f32, oh [128,15360] bf16, idx [128,80] i16,
    slots [16384] — slot id per sorted token, order [16384] — token id per
    sorted position).
    """
    order = np.argsort(cat_shard, kind="stable")
    cat_s = cat_shard[order].astype(np.int64)
    b = cat_s >> 7                                   # bucket 0..39
    bucket_start = np.searchsorted(cat_s, np.arange(N_CHUNK) * 128, side="left")
    r = np.arange(NTOK) - bucket_start[b]            # rank within bucket
    caps = np.where(b < 39, BUCKET_CAP, 128)
    main = r < caps
    n_stray = int(NTOK - main.sum())
    if n_stray > STRAY_CAP:
        raise RuntimeError(f"stray overflow: {n_stray} > {STRAY_CAP}")
    stray_rank = np.cumsum(~main) - 1
    slots = np.where(main, b * BUCKET_CAP + r, MAIN_SLOTS + stray_rank)

    token_at_slot = np.full(SLOTS, -1, np.int64)
    token_at_slot[slots] = order
    ts_mat = token_at_slot.reshape(G_TOT, 128).T     # [p, g]
    x_perm = np.ascontiguousarray(
        x_shard[np.maximum(ts_mat, 0)].astype(BF16)  # dummy slots: any row
    )

    oh = np.zeros((128, MAIN_SLOTS), dtype=ml_dtypes.float8_e4m3)
    oh[(cat_s & 127)[main], slots[main]] = 1

    vals = np.full(STRAY_CAP, -1, np.int16)          # trailing -1: gather skips
    vals[:n_stray] = cat_s[~main]
    idx = np.ascontiguousarray(
        np.tile(vals.reshape(IDX_COLS, 16).T, (8, 1)).astype(np.int16)
    )
    return x_perm, oh, idx, slots, order


RUN_KWARGS = {}  # test harness can set e.g. {"trace": True}
LAST_RESULTS = None
_NC = None


def _get_nc():
    global _NC
    if _NC is None:
        _NC = _build_nc()
    return _NC


def kernel(inputs, categories, mask_positions, table):
    global LAST_RESULTS
    inputs = np.asarray(inputs, dtype=np.float32)
    categories = np.asarray(categories).astype(np.int64)
    mask_positions = np.asarray(mask_positions).astype(np.int64)
    table = np.asarray(table, dtype=np.float32)

    # Fold both masks into the data.
    cat = categories.copy()
    cat[np.arange(B), mask_positions[:, 0]] = 0
    tbl0 = np.zeros((N_CATP, D), dtype=np.float32)
    tbl0[:N_CAT] = table
    tbl0[0] = 0.0
    tblr = tbl0.astype(BF16)
    tblc = np.ascontiguousarray(
        tblr.reshape(N_CHUNK, 128, D).transpose(1, 0, 2).reshape(128, N_CHUNK * D)
    )

    nc = _get_nc()

    in_maps = []
    posts = []
    for c in range(N_CORES):
        x_shard = inputs[c * B_PER : (c + 1) * B_PER].reshape(NTOK, D)
        cat_shard = cat[c * B_PER : (c + 1) * B_PER].reshape(NTOK)
        x_perm, oh, idx, slots, order = _prep_core(x_shard, cat_shard)
        in_maps.append(
            {"x": x_perm, "tblc": tblc, "tblr": tblr, "oh": oh, "idx": idx}
        )
        posts.append((slots, order))

    res = run_bass_kernel_spmd(
        nc, in_maps, core_ids=list(range(N_CORES)), **RUN_KWARGS
    )
    LAST_RESULTS = res

    out = np.empty((B, S, D), dtype=np.float32)
    for c in range(N_CORES):
        slots, order = posts[c]
        flat = (
            res.results[c]["out"]
            .transpose(1, 0, 2)
            .reshape(SLOTS, D)                       # slot-major [g*128+p]
        )
        shard = np.empty((NTOK, D), dtype=np.float32)
        shard[order] = flat[slots].astype(np.float32)
        out[c * B_PER : (c + 1) * B_PER] = shard.reshape(B_PER, S, D)
    return out
